# revision 4
# baseline (speedup 1.0000x reference)
"""Trainium2 Bass kernel for DirectRouting (capsule dynamic routing).

Problem: v (64, 1152, 32, 4, 4, 1), logits (1, 1152, 32, 1, 1, 1), 3 routing
iterations. Data-parallel over batch: 8 batches per NeuronCore, 8 cores.

Per-core algorithm (A=1152 on partitions as 9 tiles of 128, BJ=b*16+j=512):
  - v loaded A-major (128, 9, 512), rounded to f32r for fast PE matmuls
  - vT ((b,j)-major, (128, 4, 1152)) built on-chip via 36 PE transposes
  - iteration i: c = softmax_b(b) (free-dim ops, A-major); s via PE matmul
    (c stationary (128,32), v moving (128,512), diag extraction by mask +
    segmented reduce); squash per batch (sqrt via exp(0.5*ln) to stay in one
    ACT table set); agreement via block-diagonal P matmul over vT giving
    agr^T (b-major), transposed back on PE and added into b (A-major).
All heavy matmuls run float32r (1 cyc/col, ~1e-4 rel err); transposes fp32.
"""
import numpy as np

BATCH, A_DIM, B_DIM, PP = 64, 1152, 32, 16
N_CORES = 8
BPC = BATCH // N_CORES          # batches per core
NT = A_DIM // 128               # 9 A-tiles
BJ = B_DIM * PP                 # 512
ITERS = 3
EPS = 1e-6

_CACHE = {}


def _constants():
    I128 = np.eye(128, dtype=np.float32)
    REPM = np.zeros((128, 128), dtype=np.float32)
    for p in range(128):
        REPM[p % 16, p] = 1.0
    BM = np.zeros((128, 4, 32), dtype=np.float32)
    for b in range(32):
        for j in range(16):
            BM[(b % 8) * 16 + j, b // 8, b] = 1.0
    DM = np.zeros((32, BJ), dtype=np.float32)
    for b in range(32):
        DM[b, b * 16:(b + 1) * 16] = 1.0
    ONES = np.ones((128, 1), dtype=np.float32)
    E1 = np.zeros((128, 32), dtype=np.float32)
    E1[0, :] = 1.0
    return {"I128": I128, "REPM": REPM, "BM": BM, "DM": DM,
            "ONES": ONES, "E1": E1}


def _build():
    from contextlib import ExitStack
    import concourse.bacc as bacc
    import concourse.tile as tile
    from concourse import mybir

    F32 = mybir.dt.float32
    F32R = mybir.dt.float32r
    AF = mybir.ActivationFunctionType
    ALU = mybir.AluOpType
    X = mybir.AxisListType.X

    nc = bacc.Bacc(trn_type="TRN2")

    v_d = nc.dram_tensor("v", [BPC, A_DIM, BJ], F32, kind="ExternalInput").ap()
    lg_d = nc.dram_tensor("lg", [A_DIM, B_DIM], F32, kind="ExternalInput").ap()
    cst = {}
    for name, arr in _constants().items():
        cst[name] = nc.dram_tensor(name, list(arr.shape), F32,
                                   kind="ExternalInput").ap()
    p_d = nc.dram_tensor("p_o", [BPC, B_DIM, PP], F32, kind="ExternalOutput").ap()
    a_d = nc.dram_tensor("a_o", [BPC, B_DIM, 4], F32, kind="ExternalOutput").ap()

    with tile.TileContext(nc) as tc, ExitStack() as ctx:
        cpool = ctx.enter_context(tc.tile_pool(name="const", bufs=1))
        raw_p = ctx.enter_context(tc.tile_pool(name="raw", bufs=2))
        vr_p = ctx.enter_context(tc.tile_pool(name="vr", bufs=2))
        vt_p = ctx.enter_context(tc.tile_pool(name="vt", bufs=2))
        bt_p = ctx.enter_context(tc.tile_pool(name="bt", bufs=2))
        work = ctx.enter_context(tc.tile_pool(name="work", bufs=2))
        ps_s = ctx.enter_context(tc.tile_pool(name="ps_s", bufs=1, space="PSUM"))
        ps_a = ctx.enter_context(tc.tile_pool(name="ps_a", bufs=1, space="PSUM"))
        ps_t = ctx.enter_context(tc.tile_pool(name="ps_t", bufs=2, space="PSUM"))
        ps_m = ctx.enter_context(tc.tile_pool(name="ps_m", bufs=2, space="PSUM"))

        # ---- constants ----
        I128 = cpool.tile([128, 128], F32, tag="I128")
        nc.sync.dma_start(I128[:], cst["I128"])
        REPM = cpool.tile([128, 128], F32, tag="REPM")
        nc.sync.dma_start(REPM[:], cst["REPM"])
        BM = cpool.tile([128, 4, 32], F32, tag="BM")
        nc.sync.dma_start(BM[:], cst["BM"])
        DM = cpool.tile([32, BJ], F32, tag="DM")
        nc.sync.dma_start(DM[:], cst["DM"])
        ONES = cpool.tile([128, 1], F32, tag="ONES")
        nc.sync.dma_start(ONES[:], cst["ONES"])
        E1 = cpool.tile([128, 32], F32, tag="E1")
        nc.sync.dma_start(E1[:], cst["E1"])

        lT = cpool.tile([128, NT, B_DIM], F32, tag="lT")
        nc.sync.dma_start(lT[:], lg_d.rearrange("(t p) b -> p t b", t=NT))

        # padded scratch tiles (zero rows written once; alternation x2)
        def padpair(tag, shape, zero_from):
            ts = []
            for k in range(2):
                t = cpool.tile(list(shape), F32, tag=f"{tag}{k}")
                nc.vector.memset(t[:], 0.0)
                ts.append(t)
            return ts
        spads = padpair("spad", (128, 16), 32)
        fpads = padpair("fpad", (128, 16), 1)
        ppads = padpair("ppad", (128, 16), 32)
        pTps = padpair("pTp", (128, 32), 16)
        agps = padpair("agp", (128, A_DIM), 32)

        pfin = cpool.tile([32, BPC, PP], F32, tag="pfin")
        afin = cpool.tile([32, BPC, 4], F32, tag="afin")
        epsT = cpool.tile([1, 1], F32, tag="eps")
        nc.vector.memset(epsT[:], EPS)

        # ---- c0 = softmax(logits) over b, shared across batches ----
        e0 = work.tile([128, NT, B_DIM], F32, tag="e")
        nc.scalar.activation(e0[:], lT[:], AF.Exp)
        den0 = work.tile([128, NT], F32, tag="den")
        nc.vector.tensor_reduce(den0[:], e0[:], axis=X, op=ALU.add)
        rcp0 = work.tile([128, NT], F32, tag="rcp")
        nc.vector.reciprocal(rcp0[:], den0[:])
        c0 = cpool.tile([128, NT, B_DIM], F32R, tag="c0")
        nc.vector.scalar_tensor_tensor(
            c0[:], e0[:], 1.0,
            rcp0[:].unsqueeze(2).broadcast_to((128, NT, B_DIM)),
            ALU.mult, ALU.mult)

        # ---- per-batch pipeline ----
        for bt in range(BPC):
            raw = raw_p.tile([128, NT, BJ], F32, tag="raw")
            nc.sync.dma_start(raw[:], v_d[bt].rearrange("(t p) c -> p t c", t=NT))
            vr = vr_p.tile([128, NT, BJ], F32R, tag="vr")
            nc.vector.tensor_copy(vr[:], raw[:])
            # transposes: vT[p~, q, 128t:128t+128] = v[A=128t+p, bj=128q+p~]
            vt = vt_p.tile([128, 4, A_DIM], F32R, tag="vt")
            for t in range(NT):
                pst = ps_t.tile([128, 4, 128], F32, tag="tp")
                for q in range(4):
                    nc.tensor.transpose(pst[:, q, :],
                                        raw[:, t, 128 * q:128 * (q + 1)], I128[:])
                nc.scalar.copy(vt[:, :, 128 * t:128 * (t + 1)], pst[:])

            bT = bt_p.tile([128, NT, B_DIM], F32, tag="bT")

            for it in range(ITERS):
                bi = bt * ITERS + it
                if it == 0:
                    c_use = c0
                else:
                    e = work.tile([128, NT, B_DIM], F32, tag="e")
                    nc.scalar.activation(e[:], bT[:], AF.Exp)
                    den = work.tile([128, NT], F32, tag="den")
                    nc.vector.tensor_reduce(den[:], e[:], axis=X, op=ALU.add)
                    rcp = work.tile([128, NT], F32, tag="rcp")
                    nc.vector.reciprocal(rcp[:], den[:])
                    c_use = work.tile([128, NT, B_DIM], F32R, tag="c")
                    nc.vector.scalar_tensor_tensor(
                        c_use[:], e[:], 1.0,
                        rcp[:].unsqueeze(2).broadcast_to((128, NT, B_DIM)),
                        ALU.mult, ALU.mult)

                # step 2: s[b, bj-col] = sum_A c[A,b] v[A,bj]; diag cols wanted
                pss = ps_s.tile([32, BJ], F32, tag="s")
                for t in range(NT):
                    nc.tensor.matmul(pss[:], c_use[:, t, :], vr[:, t, :],
                                     start=(t == 0), stop=(t == NT - 1))
                m = work.tile([32, BJ], F32, tag="m")
                nc.vector.tensor_mul(m[:], pss[:], DM[:])
                spad = spads[bi % 2]
                mv = m[:].rearrange("p (b j) -> p b j", b=32).transpose([0, 2, 1])
                nc.vector.tensor_reduce(spad[0:32, :], mv, axis=X, op=ALU.add)

                # squash: ns[j] = sum_b s^2; f = sqrt(ns+eps)/(1+ns); p = f*s
                sq2 = work.tile([128, 16], F32, tag="sq2")
                nc.vector.tensor_mul(sq2[:], spad[:], spad[:])
                ns = ps_m.tile([1, 16], F32, tag="sm")
                nc.tensor.matmul(ns[:], ONES[:], sq2[:], start=True, stop=True)
                d1 = work.tile([1, 16], F32, tag="d1")
                nc.vector.tensor_scalar(d1[:], ns[:], 1.0, None, ALU.add)
                rcp2 = work.tile([1, 16], F32, tag="rcp2")
                nc.vector.reciprocal(rcp2[:], d1[:])
                lns = work.tile([1, 16], F32, tag="lns")
                nc.scalar.activation(lns[:], ns[:], AF.Ln, bias=epsT[:])
                sqv = work.tile([1, 16], F32, tag="sqv")
                nc.scalar.activation(sqv[:], lns[:], AF.Exp, scale=0.5)
                fpad = fpads[bi % 2]
                nc.vector.tensor_mul(fpad[0:1, :], sqv[:], rcp2[:])
                fbc = ps_m.tile([32, 16], F32, tag="sm")
                nc.tensor.matmul(fbc[:], E1[:], fpad[:], start=True, stop=True)
                if it == ITERS - 1:
                    nc.vector.tensor_mul(pfin[:, bt, :], spad[0:32, :], fbc[:])
                    continue
                ppad = ppads[bi % 2]
                nc.vector.tensor_mul(ppad[0:32, :], spad[0:32, :], fbc[:])

                # P_blk: PB[p~, q, b] = p[b, p~%16] * blkmask
                pt2 = ps_m.tile([16, 128], F32, tag="sm")
                nc.tensor.transpose(pt2[:], ppad[:], I128[:])
                pTp = pTps[bi % 2]
                nc.scalar.copy(pTp[0:16, :], pt2[0:16, 0:32])
                pr = ps_m.tile([128, 32], F32, tag="sm")
                nc.tensor.matmul(pr[:], REPM[:], pTp[:], start=True, stop=True)
                PB = work.tile([128, 4, 32], F32R, tag="PB")
                nc.vector.scalar_tensor_tensor(
                    PB[:], pr[:].unsqueeze(1).broadcast_to((128, 4, 32)), 1.0,
                    BM[:], ALU.mult, ALU.mult)

                # agreement: agrT[b, A] = sum_{(b,j)} PB * vT  (psum (32,3,512))
                psa = ps_a.tile([32, 3, 512], F32, tag="agr")
                for n in range(3):
                    for q in range(4):
                        nc.tensor.matmul(
                            psa[:, n, 0:384], PB[:, q, :],
                            vt[:, q, 384 * n:384 * (n + 1)],
                            start=(q == 0), stop=(q == 3))
                agp = agps[bi % 2]
                nc.scalar.copy(
                    agp[0:32, :].rearrange("p (n c) -> p n c", n=3),
                    psa[:, :, 0:384])
                # transpose agr back to A-major and add into bT
                for g, cnt in ((0, 4), (4, 4), (8, 1)):
                    pbt = ps_m.tile([128, cnt, 128], F32, tag="sm")
                    for k in range(cnt):
                        t = g + k
                        nc.tensor.transpose(pbt[:, k, :],
                                            agp[:, 128 * t:128 * (t + 1)], I128[:])
                    src0 = lT if it == 0 else bT
                    nc.vector.tensor_add(bT[:, g:g + cnt, :],
                                         src0[:, g:g + cnt, :],
                                         pbt[:, :, 0:32])

        # ---- outputs ----
        asq = work.tile([32, BPC, PP], F32, tag="asq")
        nc.vector.tensor_mul(asq[:], pfin[:], pfin[:])
        ar = work.tile([32, BPC, 4], F32, tag="ar")
        av = asq[:].rearrange("p bt (p1 p2) -> p bt p2 p1", p1=4)
        nc.vector.tensor_reduce(ar[:], av, axis=X, op=ALU.add)
        lnr = work.tile([32, BPC, 4], F32, tag="lnr")
        nc.scalar.activation(lnr[:], ar[:], AF.Ln)
        nc.scalar.activation(afin[:], lnr[:], AF.Exp, scale=0.5)
        nc.sync.dma_start(p_d.rearrange("bt b j -> b bt j"), pfin[:])
        nc.sync.dma_start(a_d.rearrange("bt b c -> b bt c"), afin[:])

    nc.compile()
    return nc


def _get_nc():
    if "nc" not in _CACHE:
        _CACHE["nc"] = _build()
    return _CACHE["nc"]


def _run(v, logits, trace=False):
    from concourse.bass_utils import run_bass_kernel_spmd
    nc = _get_nc()
    consts = _constants()
    v_flat = np.ascontiguousarray(
        v.reshape(BATCH, A_DIM, BJ).astype(np.float32))
    lg = np.ascontiguousarray(
        logits.reshape(A_DIM, B_DIM).astype(np.float32))
    in_maps = []
    for i in range(N_CORES):
        m = {"v": np.ascontiguousarray(v_flat[i * BPC:(i + 1) * BPC]),
             "lg": lg}
        m.update(consts)
        in_maps.append(m)
    res = run_bass_kernel_spmd(nc, in_maps, core_ids=list(range(N_CORES)),
                               trace=trace)
    p_full = np.concatenate([r["p_o"] for r in res.results], axis=0)
    a_full = np.concatenate([r["a_o"] for r in res.results], axis=0)
    p_out = p_full.reshape(BATCH, B_DIM, 4, 4, 1)
    a_out = a_full.reshape(BATCH, B_DIM, 4, 1)
    return (a_out, p_out), res


def kernel(a=None, v=None, logits=None, **kw):
    out, _ = _run(v, logits, trace=False)
    return out


# revision 5
# speedup vs baseline: 1.0096x; 1.0096x over previous
"""Trainium2 Bass kernel for DirectRouting (capsule dynamic routing).

Problem: v (64, 1152, 32, 4, 4, 1), logits (1, 1152, 32, 1, 1, 1), 3 routing
iterations. Data-parallel over batch: 8 batches per NeuronCore, 8 cores.

Per-core design (A=1152 on partitions as 9 tiles of 128, bj = b*16+j in 512):
  - v shipped in BOTH layouts from host: A-major (for the c.v contraction)
    and (b,j)-major vT (for the agreement contraction); both rounded on-chip
    to float32r (PE runs f32r at 1 cyc/col vs 4 for fp32, ~1e-4 rel err).
  - iteration i: c = softmax_b(b) with free-dim ops (A-major);
    s via PE matmul (c stationary (128,32), v moving (128,512)) + diagonal
    extraction (mask multiply + strided segmented reduce);
    squash with sqrt(ns) computed as bit-trick + 2 Newton steps on DVE
    (ns spans [0.7, 3e5]; avoids ACT table switching entirely);
    agreement via block-diagonal P matmul over vT giving agr^T (b-major),
    transposed back to A-major on PE and added into b.
  - outputs p (j-major) and a_out (via one selection matmul + ACT sqrt).
"""
import numpy as np

BATCH, A_DIM, B_DIM, PP = 64, 1152, 32, 16
N_CORES = 8
BPC = BATCH // N_CORES          # batches per core
NT = A_DIM // 128               # 9 A-tiles
BJ = B_DIM * PP                 # 512
ITERS = 3
EPS = 1e-6
SQRT_MAGIC = 0x1fbd1df5

_CACHE = {}


def _constants():
    I128 = np.eye(128, dtype=np.float32)
    REPM = np.zeros((128, 128), dtype=np.float32)
    for p in range(128):
        REPM[p % 16, p] = 1.0
    BM = np.zeros((128, 4, 32), dtype=np.float32)
    for b in range(32):
        for j in range(16):
            BM[(b % 8) * 16 + j, b // 8, b] = 1.0
    DM = np.zeros((32, BJ), dtype=np.float32)
    for b in range(32):
        DM[b, b * 16:(b + 1) * 16] = 1.0
    ONES = np.ones((128, 1), dtype=np.float32)
    S4 = np.zeros((128, 4), dtype=np.float32)
    for j in range(16):
        S4[j, j % 4] = 1.0
    return {"I128": I128, "REPM": REPM, "BM": BM, "DM": DM,
            "ONES": ONES, "S4": S4}


def _build():
    from contextlib import ExitStack
    import concourse.bacc as bacc
    import concourse.tile as tile
    from concourse import mybir

    F32 = mybir.dt.float32
    F32R = mybir.dt.float32r
    I32 = mybir.dt.int32
    AF = mybir.ActivationFunctionType
    ALU = mybir.AluOpType
    X = mybir.AxisListType.X

    nc = bacc.Bacc(trn_type="TRN2")

    v_d = nc.dram_tensor("v", [BPC, A_DIM, BJ], F32, kind="ExternalInput").ap()
    vt_d = nc.dram_tensor("vt", [BPC, BJ, A_DIM], F32, kind="ExternalInput").ap()
    lg_d = nc.dram_tensor("lg", [A_DIM, B_DIM], F32, kind="ExternalInput").ap()
    cst = {}
    for name, arr in _constants().items():
        cst[name] = nc.dram_tensor(name, list(arr.shape), F32,
                                   kind="ExternalInput").ap()
    p_d = nc.dram_tensor("p_o", [BPC, B_DIM, PP], F32, kind="ExternalOutput").ap()
    a_d = nc.dram_tensor("a_o", [BPC, B_DIM, 4], F32, kind="ExternalOutput").ap()

    with tile.TileContext(nc) as tc, ExitStack() as ctx:
        cpool = ctx.enter_context(tc.tile_pool(name="const", bufs=1))
        raw_p = ctx.enter_context(tc.tile_pool(name="raw", bufs=2))
        rawt_p = ctx.enter_context(tc.tile_pool(name="rawt", bufs=2))
        vr_p = ctx.enter_context(tc.tile_pool(name="vr", bufs=2))
        vt_p = ctx.enter_context(tc.tile_pool(name="vt", bufs=2))
        bt_p = ctx.enter_context(tc.tile_pool(name="bt", bufs=2))
        work = ctx.enter_context(tc.tile_pool(name="work", bufs=3))
        ps_s = ctx.enter_context(tc.tile_pool(name="ps_s", bufs=2, space="PSUM"))
        ps_a = ctx.enter_context(tc.tile_pool(name="ps_a", bufs=1, space="PSUM"))
        ps_m = ctx.enter_context(tc.tile_pool(name="ps_m", bufs=3, space="PSUM"))

        # ---- constants ----
        I128 = cpool.tile([128, 128], F32, tag="I128")
        nc.sync.dma_start(I128[:], cst["I128"])
        REPM = cpool.tile([128, 128], F32, tag="REPM")
        nc.sync.dma_start(REPM[:], cst["REPM"])
        BM = cpool.tile([128, 4, 32], F32, tag="BM")
        nc.sync.dma_start(BM[:], cst["BM"])
        DM = cpool.tile([32, BJ], F32, tag="DM")
        nc.sync.dma_start(DM[:], cst["DM"])
        ONES = cpool.tile([128, 1], F32, tag="ONES")
        nc.sync.dma_start(ONES[:], cst["ONES"])
        S4 = cpool.tile([128, 4], F32, tag="S4")
        nc.sync.dma_start(S4[:], cst["S4"])

        lT = cpool.tile([128, NT, B_DIM], F32, tag="lT")
        nc.sync.dma_start(lT[:], lg_d.rearrange("(t p) b -> p t b", t=NT))

        # padded scratch (zeros persist in never-written rows; x2 alternation)
        def padpair(tag, shape):
            ts_ = []
            for k in range(2):
                t = cpool.tile(list(shape), F32, tag=f"{tag}{k}")
                nc.vector.memset(t[:], 0.0)
                ts_.append(t)
            return ts_
        spads = padpair("spad", (128, 16))
        pTps = padpair("pTp", (128, 32))
        agps = padpair("agp", (128, A_DIM))

        pfinT = cpool.tile([16, BPC, B_DIM], F32, tag="pfinT")
        asqp = cpool.tile([128, BPC, B_DIM], F32, tag="asqp")
        nc.vector.memset(asqp[:], 0.0)
        afinT = cpool.tile([4, BPC, B_DIM], F32, tag="afinT")

        # ---- c0 = softmax(logits) over b, shared across batches ----
        e0 = work.tile([128, NT, B_DIM], F32, tag="e")
        nc.scalar.activation(e0[:], lT[:], AF.Exp)
        den0 = work.tile([128, NT], F32, tag="den")
        nc.vector.tensor_reduce(den0[:], e0[:], axis=X, op=ALU.add)
        rcp0 = work.tile([128, NT], F32, tag="rcp")
        nc.vector.reciprocal(rcp0[:], den0[:])
        c0 = cpool.tile([128, NT, B_DIM], F32R, tag="c0")
        nc.vector.scalar_tensor_tensor(
            c0[:], e0[:], 1.0,
            rcp0[:].unsqueeze(2).broadcast_to((128, NT, B_DIM)),
            ALU.mult, ALU.mult)

        # ---- per-batch pipeline ----
        for bt in range(BPC):
            raw = raw_p.tile([128, NT, BJ], F32, tag="raw")
            nc.sync.dma_start(raw[:], v_d[bt].rearrange("(t p) c -> p t c", t=NT))
            vr = vr_p.tile([128, NT, BJ], F32R, tag="vr")
            nc.scalar.copy(vr[:], raw[:])
            rawt = rawt_p.tile([128, 4, A_DIM], F32, tag="rawt")
            nc.sync.dma_start(rawt[:],
                              vt_d[bt].rearrange("(q pp) a -> pp q a", q=4))
            vt = vt_p.tile([128, 4, A_DIM], F32R, tag="vt")
            nc.vector.tensor_copy(vt[:], rawt[:])

            bT = bt_p.tile([128, NT, B_DIM], F32, tag="bT")

            for it in range(ITERS):
                bi = bt * ITERS + it
                if it == 0:
                    c_use = c0
                else:
                    e = work.tile([128, NT, B_DIM], F32, tag="e")
                    nc.scalar.activation(e[:], bT[:], AF.Exp)
                    den = work.tile([128, NT], F32, tag="den")
                    nc.vector.tensor_reduce(den[:], e[:], axis=X, op=ALU.add)
                    rcp = work.tile([128, NT], F32, tag="rcp")
                    nc.vector.reciprocal(rcp[:], den[:])
                    c_use = work.tile([128, NT, B_DIM], F32R, tag="c")
                    nc.vector.scalar_tensor_tensor(
                        c_use[:], e[:], 1.0,
                        rcp[:].unsqueeze(2).broadcast_to((128, NT, B_DIM)),
                        ALU.mult, ALU.mult)

                # step 2: s outer product + diagonal extraction
                pss = ps_s.tile([32, BJ], F32, tag="s")
                for t in range(NT):
                    nc.tensor.matmul(pss[:], c_use[:, t, :], vr[:, t, :],
                                     start=(t == 0), stop=(t == NT - 1))
                m = work.tile([32, BJ], F32, tag="m")
                nc.vector.tensor_mul(m[:], pss[:], DM[:])
                spad = spads[bi % 2]
                mv = m[:].rearrange("p (b j) -> p b j", b=32).transpose([0, 2, 1])
                nc.vector.tensor_reduce(spad[0:32, :], mv, axis=X, op=ALU.add)

                # ns[j] = sum_b s^2 -> (16,1) j-major via matmul
                sq2 = work.tile([128, 16], F32, tag="sq2")
                nc.vector.tensor_mul(sq2[:], spad[:], spad[:])
                nsp = ps_m.tile([16, 1], F32, tag="sm")
                nc.tensor.matmul(nsp[:], sq2[:], ONES[:], start=True, stop=True)
                xs = work.tile([16, 1], F32, tag="xs")
                nc.vector.tensor_scalar(xs[:], nsp[:], 0.0, None, ALU.add)
                # sqrt(ns): bit trick + 2 Newton steps (all DVE)
                ti1 = work.tile([16, 1], I32, tag="ti1")
                nc.vector.tensor_scalar(ti1[:], xs[:].bitcast(I32), 1, None,
                                        ALU.logical_shift_right)
                ti2 = work.tile([16, 1], I32, tag="ti2")
                nc.vector.tensor_scalar(ti2[:], ti1[:], SQRT_MAGIC, None, ALU.add)
                y = ti2[:].bitcast(F32)
                for st in range(2):
                    r_ = work.tile([16, 1], F32, tag=f"nr{st}")
                    nc.vector.reciprocal(r_[:], y)
                    q_ = work.tile([16, 1], F32, tag=f"nq{st}")
                    nc.vector.tensor_mul(q_[:], xs[:], r_[:])
                    w_ = work.tile([16, 1], F32, tag=f"nw{st}")
                    nc.vector.tensor_add(w_[:], q_[:], y)
                    y2 = work.tile([16, 1], F32, tag=f"ny{st}")
                    nc.vector.tensor_scalar(y2[:], w_[:], 0.5, None, ALU.mult)
                    y = y2[:]
                # f = sqrt(ns+eps)/(1+ns)  (eps negligible vs ns >= ~0.7)
                d1 = work.tile([16, 1], F32, tag="d1")
                nc.vector.tensor_scalar(d1[:], xs[:], 1.0, None, ALU.add)
                rc = work.tile([16, 1], F32, tag="rc")
                nc.vector.reciprocal(rc[:], d1[:])
                f_ = work.tile([16, 1], F32, tag="f_")
                nc.vector.tensor_mul(f_[:], y, rc[:])

                # s^T via PE transpose; p^T = f * s^T (per-partition scalar)
                sTp = ps_m.tile([16, 128], F32, tag="sm")
                nc.tensor.transpose(sTp[:], spad[:], I128[:])
                if it == ITERS - 1:
                    nc.vector.tensor_scalar(pfinT[:, bt, :], sTp[0:16, 0:32],
                                            f_[:], None, ALU.mult)
                    continue
                pTp = pTps[bi % 2]
                nc.vector.tensor_scalar(pTp[0:16, :], sTp[0:16, 0:32],
                                        f_[:], None, ALU.mult)
                pr = ps_m.tile([128, 32], F32, tag="sm")
                nc.tensor.matmul(pr[:], REPM[:], pTp[:], start=True, stop=True)
                PB = work.tile([128, 4, 32], F32R, tag="PB")
                nc.vector.scalar_tensor_tensor(
                    PB[:], pr[:].unsqueeze(1).broadcast_to((128, 4, 32)), 1.0,
                    BM[:], ALU.mult, ALU.mult)

                # agreement: agrT (32, 1152) in 3 psum chunks
                psa = ps_a.tile([32, 3, 512], F32, tag="agr")
                for n in range(3):
                    for q in range(4):
                        nc.tensor.matmul(
                            psa[:, n, 0:384], PB[:, q, :],
                            vt[:, q, 384 * n:384 * (n + 1)],
                            start=(q == 0), stop=(q == 3))
                agp = agps[bi % 2]
                nc.scalar.copy(
                    agp[0:32, :].rearrange("p (n c) -> p n c", n=3),
                    psa[:, :, 0:384])
                # transpose agr back to A-major, add into bT
                for g, cnt in ((0, 4), (4, 4), (8, 1)):
                    pbt = ps_m.tile([128, cnt, 128], F32, tag="sm")
                    for k in range(cnt):
                        t = g + k
                        nc.tensor.transpose(pbt[:, k, :],
                                            agp[:, 128 * t:128 * (t + 1)], I128[:])
                    src0 = lT if it == 0 else bT
                    nc.vector.tensor_add(bT[:, g:g + cnt, :],
                                         src0[:, g:g + cnt, :],
                                         pbt[:, :, 0:32])

        # ---- outputs ----
        nc.vector.tensor_mul(asqp[0:16, :], pfinT[:], pfinT[:])
        ap_ = ps_m.tile([4, BPC * B_DIM], F32, tag="sm")
        nc.tensor.matmul(ap_[:], S4[:],
                         asqp[:].rearrange("p bt b -> p (bt b)"),
                         start=True, stop=True)
        nc.scalar.activation(afinT[:].rearrange("p bt b -> p (bt b)"), ap_[:],
                             AF.Sqrt)
        nc.sync.dma_start(p_d.rearrange("bt b j -> j bt b"), pfinT[:])
        nc.sync.dma_start(a_d.rearrange("bt b c -> c bt b"), afinT[:])

    nc.compile()
    return nc


def _get_nc():
    if "nc" not in _CACHE:
        _CACHE["nc"] = _build()
    return _CACHE["nc"]


def _run(v, logits, trace=False):
    from concourse.bass_utils import run_bass_kernel_spmd
    nc = _get_nc()
    consts = _constants()
    v_flat = np.ascontiguousarray(
        v.reshape(BATCH, A_DIM, BJ).astype(np.float32))
    vt_flat = np.ascontiguousarray(v_flat.transpose(0, 2, 1))
    lg = np.ascontiguousarray(
        logits.reshape(A_DIM, B_DIM).astype(np.float32))
    in_maps = []
    for i in range(N_CORES):
        m = {"v": np.ascontiguousarray(v_flat[i * BPC:(i + 1) * BPC]),
             "vt": np.ascontiguousarray(vt_flat[i * BPC:(i + 1) * BPC]),
             "lg": lg}
        m.update(consts)
        in_maps.append(m)
    res = run_bass_kernel_spmd(nc, in_maps, core_ids=list(range(N_CORES)),
                               trace=trace)
    # p_o is (BPC, B, PP); a_o is (BPC, B, 4)
    p_full = np.concatenate([r["p_o"] for r in res.results], axis=0)
    a_full = np.concatenate([r["a_o"] for r in res.results], axis=0)
    p_out = p_full.reshape(BATCH, B_DIM, 4, 4, 1)
    a_out = a_full.reshape(BATCH, B_DIM, 4, 1)
    return (a_out, p_out), res


def kernel(a=None, v=None, logits=None, **kw):
    out, _ = _run(v, logits, trace=False)
    return out


# revision 7
# speedup vs baseline: 1.4503x; 1.4365x over previous
"""Trainium2 Bass kernel for DirectRouting (capsule dynamic routing).

Problem: v (64, 1152, 32, 4, 4, 1), logits (1, 1152, 32, 1, 1, 1), 3 routing
iterations. Data-parallel over batch: 8 batches per NeuronCore, 8 cores.

Per-core design (A=1152 on partitions as 9 tiles of 128, bj = b*16+j in 512):
  - v shipped in BOTH layouts from host: A-major (for the c.v contraction)
    and (b,j)-major vT (for the agreement contraction); both rounded on-chip
    to float32r (PE runs f32r at 1 cyc/col vs 4 for fp32, ~1e-4 rel err).
  - iteration i: c = softmax_b(b) with free-dim ops (A-major);
    s via PE matmul (c stationary (128,32), v moving (128,512)) + diagonal
    extraction (mask multiply + strided segmented reduce);
    squash with sqrt(ns) computed as bit-trick + 2 Newton steps on DVE
    (ns spans [0.7, 3e5]; avoids ACT table switching entirely);
    agreement via block-diagonal P matmul over vT giving agr^T (b-major),
    transposed back to A-major on PE and added into b.
  - outputs p (j-major) and a_out (via one selection matmul + ACT sqrt).
"""
import numpy as np

BATCH, A_DIM, B_DIM, PP = 64, 1152, 32, 16
N_CORES = 8
BPC = BATCH // N_CORES          # batches per core
NT = A_DIM // 128               # 9 A-tiles
BJ = B_DIM * PP                 # 512
ITERS = 3
EPS = 1e-6
SQRT_MAGIC = 0x1fbd1df5

_CACHE = {}


def _constants():
    I128 = np.eye(128, dtype=np.float32)
    REPM = np.zeros((128, 128), dtype=np.float32)
    for p in range(128):
        REPM[p % 16, p] = 1.0
    BM = np.zeros((128, 4, 32), dtype=np.float32)
    for b in range(32):
        for j in range(16):
            BM[(b % 8) * 16 + j, b // 8, b] = 1.0
    DM = np.zeros((32, BJ), dtype=np.float32)
    for b in range(32):
        DM[b, b * 16:(b + 1) * 16] = 1.0
    ONES = np.ones((128, 1), dtype=np.float32)
    S4 = np.zeros((128, 4), dtype=np.float32)
    for j in range(16):
        S4[j, j % 4] = 1.0
    return {"I128": I128, "REPM": REPM, "BM": BM, "DM": DM,
            "ONES": ONES, "S4": S4}


def _build():
    from contextlib import ExitStack
    import concourse.bacc as bacc
    import concourse.tile as tile
    from concourse import mybir

    F32 = mybir.dt.float32
    F32R = mybir.dt.float32r
    I32 = mybir.dt.int32
    AF = mybir.ActivationFunctionType
    ALU = mybir.AluOpType
    X = mybir.AxisListType.X

    nc = bacc.Bacc(trn_type="TRN2")

    v_d = nc.dram_tensor("v", [BPC, A_DIM, BJ], F32, kind="ExternalInput").ap()
    vt_d = nc.dram_tensor("vt", [BPC, BJ, A_DIM], F32, kind="ExternalInput").ap()
    lg_d = nc.dram_tensor("lg", [A_DIM, B_DIM], F32, kind="ExternalInput").ap()
    cst = {}
    for name, arr in _constants().items():
        cst[name] = nc.dram_tensor(name, list(arr.shape), F32,
                                   kind="ExternalInput").ap()
    p_d = nc.dram_tensor("p_o", [BPC, B_DIM, PP], F32, kind="ExternalOutput").ap()
    a_d = nc.dram_tensor("a_o", [BPC, B_DIM, 4], F32, kind="ExternalOutput").ap()

    with tile.TileContext(nc) as tc, ExitStack() as ctx:
        cpool = ctx.enter_context(tc.tile_pool(name="const", bufs=1))
        raw_p = ctx.enter_context(tc.tile_pool(name="raw", bufs=1))
        rawt_p = ctx.enter_context(tc.tile_pool(name="rawt", bufs=1))
        vr_p = ctx.enter_context(tc.tile_pool(name="vr", bufs=3))
        vt_p = ctx.enter_context(tc.tile_pool(name="vt", bufs=3))
        bt_p = ctx.enter_context(tc.tile_pool(name="bt", bufs=2))
        work = ctx.enter_context(tc.tile_pool(name="work", bufs=4))
        ps_s = ctx.enter_context(tc.tile_pool(name="ps_s", bufs=2, space="PSUM"))
        ps_a = ctx.enter_context(tc.tile_pool(name="ps_a", bufs=1, space="PSUM"))
        ps_m = ctx.enter_context(tc.tile_pool(name="ps_m", bufs=3, space="PSUM"))

        # ---- constants ----
        I128 = cpool.tile([128, 128], F32, tag="I128")
        nc.sync.dma_start(I128[:], cst["I128"])
        REPM = cpool.tile([128, 128], F32, tag="REPM")
        nc.sync.dma_start(REPM[:], cst["REPM"])
        BM = cpool.tile([128, 4, 32], F32, tag="BM")
        nc.sync.dma_start(BM[:], cst["BM"])
        DM = cpool.tile([32, BJ], F32, tag="DM")
        nc.sync.dma_start(DM[:], cst["DM"])
        ONES = cpool.tile([128, 1], F32, tag="ONES")
        nc.sync.dma_start(ONES[:], cst["ONES"])
        S4 = cpool.tile([128, 4], F32, tag="S4")
        nc.sync.dma_start(S4[:], cst["S4"])

        lT = cpool.tile([128, NT, B_DIM], F32, tag="lT")
        nc.sync.dma_start(lT[:], lg_d.rearrange("(t p) b -> p t b", t=NT))

        # padded scratch (zeros persist in never-written rows; x2 alternation)
        def padpair(tag, shape):
            ts_ = []
            for k in range(2):
                t = cpool.tile(list(shape), F32, tag=f"{tag}{k}")
                nc.vector.memset(t[:], 0.0)
                ts_.append(t)
            return ts_
        spads = padpair("spad", (128, 16))
        pTps = padpair("pTp", (128, 32))
        agps = padpair("agp", (128, A_DIM))

        pfinT = cpool.tile([16, BPC, B_DIM], F32, tag="pfinT")
        asqp = cpool.tile([128, BPC, B_DIM], F32, tag="asqp")
        nc.vector.memset(asqp[:], 0.0)
        afinT = cpool.tile([4, BPC, B_DIM], F32, tag="afinT")

        # ---- c0 = softmax(logits) over b, shared across batches ----
        e0 = work.tile([128, NT, B_DIM], F32, tag="e")
        nc.scalar.activation(e0[:], lT[:], AF.Exp)
        den0 = work.tile([128, NT], F32, tag="den")
        nc.vector.tensor_reduce(den0[:], e0[:], axis=X, op=ALU.add)
        rcp0 = work.tile([128, NT], F32, tag="rcp")
        nc.vector.reciprocal(rcp0[:], den0[:])
        c0 = cpool.tile([128, NT, B_DIM], F32R, tag="c0")
        nc.vector.scalar_tensor_tensor(
            c0[:], e0[:], 1.0,
            rcp0[:].unsqueeze(2).broadcast_to((128, NT, B_DIM)),
            ALU.mult, ALU.mult)

        # ---- per-batch helpers (emission interleaved across a batch pair) ----
        state = {}

        def load_batch(bt):
            raw = raw_p.tile([128, NT, BJ], F32, tag="raw")
            nc.sync.dma_start(raw[:], v_d[bt].rearrange("(t p) c -> p t c", t=NT))
            vr = vr_p.tile([128, NT, BJ], F32R, tag="vr")
            nc.scalar.copy(vr[:], raw[:])
            rawt = rawt_p.tile([128, 4, A_DIM], F32, tag="rawt")
            nc.sync.dma_start(rawt[:],
                              vt_d[bt].rearrange("(q pp) a -> pp q a", q=4))
            vt = vt_p.tile([128, 4, A_DIM], F32R, tag="vt")
            nc.vector.tensor_copy(vt[:], rawt[:])
            bT = bt_p.tile([128, NT, B_DIM], F32, tag="bT")
            state[bt] = {"vr": vr, "vt": vt, "bT": bT}

        def softmax(bt, it):
            st_ = state[bt]
            if it == 0:
                st_["c"] = c0
                return
            bT = st_["bT"]
            e = work.tile([128, NT, B_DIM], F32, tag="e")
            nc.scalar.activation(e[:], bT[:], AF.Exp)
            den = work.tile([128, NT], F32, tag="den")
            nc.vector.tensor_reduce(den[:], e[:], axis=X, op=ALU.add)
            rcp = work.tile([128, NT], F32, tag="rcp")
            nc.vector.reciprocal(rcp[:], den[:])
            c_use = work.tile([128, NT, B_DIM], F32R, tag="c")
            nc.vector.scalar_tensor_tensor(
                c_use[:], e[:], 1.0,
                rcp[:].unsqueeze(2).broadcast_to((128, NT, B_DIM)),
                ALU.mult, ALU.mult)
            st_["c"] = c_use

        def step2(bt):
            st_ = state[bt]
            pss = ps_s.tile([32, BJ], F32, tag="s")
            for t in range(NT):
                nc.tensor.matmul(pss[:], st_["c"][:, t, :], st_["vr"][:, t, :],
                                 start=(t == 0), stop=(t == NT - 1))
            st_["pss"] = pss

        def diag(bt):
            st_ = state[bt]
            m = work.tile([32, BJ], F32, tag="m")
            nc.vector.tensor_mul(m[:], st_["pss"][:], DM[:])
            spad = spads[bt % 2]
            mv = m[:].rearrange("p (b j) -> p b j", b=32).transpose([0, 2, 1])
            nc.vector.tensor_reduce(spad[0:32, :], mv, axis=X, op=ALU.add)
            st_["spad"] = spad

        def squash(bt, it):
            st_ = state[bt]
            spad = st_["spad"]
            sq2 = work.tile([128, 16], F32, tag="sq2")
            nc.vector.tensor_mul(sq2[:], spad[:], spad[:])
            nsp = ps_m.tile([16, 1], F32, tag="sm")
            nc.tensor.matmul(nsp[:], sq2[:], ONES[:], start=True, stop=True)
            xs = work.tile([16, 1], F32, tag="xs")
            nc.vector.tensor_scalar(xs[:], nsp[:], 0.0, None, ALU.add)
            # sqrt(ns): bit trick + 2 Newton steps (all DVE)
            ti1 = work.tile([16, 1], I32, tag="ti1")
            nc.vector.tensor_scalar(ti1[:], xs[:].bitcast(I32), 1, None,
                                    ALU.logical_shift_right)
            ti2 = work.tile([16, 1], I32, tag="ti2")
            nc.vector.tensor_scalar(ti2[:], ti1[:], SQRT_MAGIC, None, ALU.add)
            y = ti2[:].bitcast(F32)
            for st in range(2):
                r_ = work.tile([16, 1], F32, tag=f"nr{st}")
                nc.vector.reciprocal(r_[:], y)
                q_ = work.tile([16, 1], F32, tag=f"nq{st}")
                nc.vector.tensor_mul(q_[:], xs[:], r_[:])
                w_ = work.tile([16, 1], F32, tag=f"nw{st}")
                nc.vector.tensor_add(w_[:], q_[:], y)
                y2 = work.tile([16, 1], F32, tag=f"ny{st}")
                nc.vector.tensor_scalar(y2[:], w_[:], 0.5, None, ALU.mult)
                y = y2[:]
            # f = sqrt(ns+eps)/(1+ns)  (eps negligible vs ns >= ~0.7)
            d1 = work.tile([16, 1], F32, tag="d1")
            nc.vector.tensor_scalar(d1[:], xs[:], 1.0, None, ALU.add)
            rc = work.tile([16, 1], F32, tag="rc")
            nc.vector.reciprocal(rc[:], d1[:])
            f_ = work.tile([16, 1], F32, tag="f_")
            nc.vector.tensor_mul(f_[:], y, rc[:])
            # s^T via PE transpose; p^T = f * s^T (per-partition scalar)
            sTp = ps_m.tile([16, 128], F32, tag="sm")
            nc.tensor.transpose(sTp[:], spad[:], I128[:])
            if it == ITERS - 1:
                nc.vector.tensor_scalar(pfinT[:, bt, :], sTp[0:16, 0:32],
                                        f_[:], None, ALU.mult)
                return
            pTp = pTps[bt % 2]
            nc.vector.tensor_scalar(pTp[0:16, :], sTp[0:16, 0:32],
                                    f_[:], None, ALU.mult)
            pr = ps_m.tile([128, 32], F32, tag="sm")
            nc.tensor.matmul(pr[:], REPM[:], pTp[:], start=True, stop=True)
            PB = work.tile([128, 4, 32], F32R, tag="PB")
            nc.vector.scalar_tensor_tensor(
                PB[:], pr[:].unsqueeze(1).broadcast_to((128, 4, 32)), 1.0,
                BM[:], ALU.mult, ALU.mult)
            st_["PB"] = PB

        def agree(bt):
            st_ = state[bt]
            psa = ps_a.tile([32, 3, 512], F32, tag="agr")
            for n in range(3):
                for q in range(4):
                    nc.tensor.matmul(
                        psa[:, n, 0:384], st_["PB"][:, q, :],
                        st_["vt"][:, q, 384 * n:384 * (n + 1)],
                        start=(q == 0), stop=(q == 3))
            st_["psa"] = psa

        def b_update(bt, it):
            st_ = state[bt]
            bT = st_["bT"]
            agp = agps[bt % 2]
            nc.scalar.copy(
                agp[0:32, :].rearrange("p (n c) -> p n c", n=3),
                st_["psa"][:, :, 0:384])
            for g, cnt in ((0, 4), (4, 4), (8, 1)):
                pbt = ps_m.tile([128, cnt, 128], F32, tag="sm")
                for k in range(cnt):
                    t = g + k
                    nc.tensor.transpose(pbt[:, k, :],
                                        agp[:, 128 * t:128 * (t + 1)], I128[:])
                src0 = lT if it == 0 else bT
                nc.vector.tensor_add(bT[:, g:g + cnt, :],
                                     src0[:, g:g + cnt, :],
                                     pbt[:, :, 0:32])

        # paired-batch pipeline: emit the two batches' phases interleaved so
        # one batch's matmuls fill the other's serial DVE/ACT chains
        for pair in range(BPC // 2):
            A, B = 2 * pair, 2 * pair + 1
            load_batch(A)
            load_batch(B)
            for it in range(ITERS):
                for bt in (A, B):
                    softmax(bt, it)
                for bt in (A, B):
                    step2(bt)
                for bt in (A, B):
                    diag(bt)
                for bt in (A, B):
                    squash(bt, it)
                if it < ITERS - 1:
                    for bt in (A, B):
                        agree(bt)
                    for bt in (A, B):
                        b_update(bt, it)
            state.pop(A)
            state.pop(B)

        # ---- outputs ----
        nc.vector.tensor_mul(asqp[0:16, :], pfinT[:], pfinT[:])
        ap_ = ps_m.tile([4, BPC * B_DIM], F32, tag="sm")
        nc.tensor.matmul(ap_[:], S4[:],
                         asqp[:].rearrange("p bt b -> p (bt b)"),
                         start=True, stop=True)
        nc.scalar.activation(afinT[:].rearrange("p bt b -> p (bt b)"), ap_[:],
                             AF.Sqrt)
        nc.sync.dma_start(p_d.rearrange("bt b j -> j bt b"), pfinT[:])
        nc.sync.dma_start(a_d.rearrange("bt b c -> c bt b"), afinT[:])

    nc.compile()
    return nc


def _get_nc():
    if "nc" not in _CACHE:
        _CACHE["nc"] = _build()
    return _CACHE["nc"]


def _run(v, logits, trace=False):
    from concourse.bass_utils import run_bass_kernel_spmd
    nc = _get_nc()
    consts = _constants()
    v_flat = np.ascontiguousarray(
        v.reshape(BATCH, A_DIM, BJ).astype(np.float32))
    vt_flat = np.ascontiguousarray(v_flat.transpose(0, 2, 1))
    lg = np.ascontiguousarray(
        logits.reshape(A_DIM, B_DIM).astype(np.float32))
    in_maps = []
    for i in range(N_CORES):
        m = {"v": np.ascontiguousarray(v_flat[i * BPC:(i + 1) * BPC]),
             "vt": np.ascontiguousarray(vt_flat[i * BPC:(i + 1) * BPC]),
             "lg": lg}
        m.update(consts)
        in_maps.append(m)
    res = run_bass_kernel_spmd(nc, in_maps, core_ids=list(range(N_CORES)),
                               trace=trace)
    # p_o is (BPC, B, PP); a_o is (BPC, B, 4)
    p_full = np.concatenate([r["p_o"] for r in res.results], axis=0)
    a_full = np.concatenate([r["a_o"] for r in res.results], axis=0)
    p_out = p_full.reshape(BATCH, B_DIM, 4, 4, 1)
    a_out = a_full.reshape(BATCH, B_DIM, 4, 1)
    return (a_out, p_out), res


def kernel(a=None, v=None, logits=None, **kw):
    out, _ = _run(v, logits, trace=False)
    return out
